# revision 1
# baseline (speedup 1.0000x reference)
"""VQ codebook argmax kernel for Trainium2 (8 NeuronCores, SPMD data-parallel).

Problem: x [2,96,48,48,48] fp32, prototypes [512,96] fp32.
Output: argmax_k cosine_sim(x[:, :, v], prototypes[k]) -> [2,48,48,48] int32.

Math notes:
  - argmax over k of (x_hat . p_hat_k) == argmax over k of (x . p_hat_k):
    per-voxel positive scaling (1/||x||) never changes the argmax, so x is
    NOT normalized (saves a full partition-dim reduction on device).
  - prototypes ARE normalized (host side, fp32, same formula as reference).
  - matmul precision: sims = xh@Ph + xl@Ph + xh@Pl with xh=bf16(x),
    xl=bf16(x-xh), Ph=bf16(pn), Pl=bf16(pn-Ph). Error ~2^-17 relative; the
    measured top-2 sim gap distribution makes this produce 0 argmax flips
    vs the fp32 reference (verified offline on the actual input).
  - argmax on device: single fused custom DVE op per 128-voxel tile.
    The 512 sims live in one PSUM bank [128, 512]; columns are permuted so
    column q holds proto 511-2q and column 256+q holds proto 510-2q.
    The op consumes two 256-wide streams (Src0 = cols 0:256 from PSUM,
    Src1 = cols 256:512 via an SBUF copy done by the Scalar engine) and
    folds: m = max(a,b); rec = (m == running_max(m)); wo = (m == b);
    pos = (2(j+1) - 1024) + wo; accum MAX of select(rec, pos, -FLT_MAX).
    The accumulated A encodes the winner: k* = -(A + 511), and the
    (j asc, wo) priority order makes ties resolve EXACTLY like np.argmax
    (first occurrence) - fuzz-verified 20000 cases.
"""

import numpy as np
import ml_dtypes
from contextlib import ExitStack

import concourse.bass as bass
import concourse.bacc as bacc
import concourse.tile as tile
from concourse import mybir
from concourse.bass_utils import run_bass_kernel_spmd

# ----------------------------------------------------------------------------
# problem constants (hardcoded per contract)
N_CORES = 8
B, C, D, H, W = 2, 96, 48, 48, 48
N_VOX = B * D * H * W            # 221184
VOX_PER_CORE = N_VOX // N_CORES  # 27648
K = 512                          # prototypes
TILE_V = 128                     # voxels per matmul tile (PSUM partition dim)
TILES_PER_CORE = VOX_PER_CORE // TILE_V  # 216
CHUNK_V = 1024                   # voxels per DMA chunk
CHUNKS = VOX_PER_CORE // CHUNK_V  # 27
TILES_PER_CHUNK = CHUNK_V // TILE_V  # 8

_BF16 = ml_dtypes.bfloat16

# ----------------------------------------------------------------------------
# custom DVE op registration (argmax fold over paired streams)

_VQARG_NAME = "VQ_ARGMAX_ANT"
_VQARG_OP = None


def _vqarg_reference(in0, in1, c0, c1, c2):
    a = np.asarray(in0, np.float32)
    b = np.asarray(in1, np.float32)
    p = a.shape[0]
    a2 = a.reshape(p, -1)
    b2 = b.reshape(p, -1)
    c1v = float(c1) if np.isscalar(c1) or isinstance(c1, float) else np.asarray(c1, np.float32)
    m = np.maximum(a2, b2)
    r = np.maximum.accumulate(m, axis=1)
    rec = m == r
    wo = (m == b2).astype(np.float32)
    n = a2.shape[1]
    s2 = (np.float32(-float(c2)) + np.float32(c1v) * np.arange(1, n + 1, dtype=np.float32))
    pos = s2[None, :] + wo
    body = np.where(rec, pos, np.float32(-3.4028235e38)).astype(np.float32)
    acc = body.max(axis=1, keepdims=True)
    return body.reshape(a.shape), acc


def _register_vqarg():
    global _VQARG_OP
    if _VQARG_OP is not None:
        return _VQARG_OP
    from concourse.dve_spec import (
        Spec, Src0, Src1, C1, C2, Zero, MaxNeg, eq, select, scan, AluOp, maxx,
        lower, _has_src1 as has_src1,
    )
    from concourse import dve_ops
    from concourse.dve_uop import DveOpSpec

    m = maxx(Src0, Src1)
    r = scan(AluOp.MAX, m)
    rec = eq(m, r)
    wo = eq(m, Src1)
    s2 = scan(AluOp.ADD, C1, init=Zero - C2)
    pos = s2 + wo
    spec = Spec(
        body=select(rec, pos, MaxNeg),
        accum=AluOp.MAX,
        reference=_vqarg_reference,
    )

    if _VQARG_NAME in dve_ops._SUB_OPCODE_FOR_NAME:
        row = dve_ops._SUB_OPCODE_FOR_NAME[_VQARG_NAME]
    else:
        row = max(dve_ops._SUB_OPCODE_FOR_NAME.values()) + 1
        assert row < 0x20, "no free custom-DVE opcode row"
        dve_ops._SUB_OPCODE_FOR_NAME[_VQARG_NAME] = row

    shas = {}
    for ver in ("v3", "v4"):
        s = DveOpSpec(
            name=_VQARG_NAME,
            opcode=row,
            uops=lower(spec, ver=ver),
            rd1_en=has_src1(spec),
        )
        shas[ver] = s.sha(ver)

    op = dve_ops.DveOp(_VQARG_NAME, spec, subdim=False, uops_sha=shas)
    if all(o.name != _VQARG_NAME for o in dve_ops.OPS):
        dve_ops.OPS.append(op)
    dve_ops.CUSTOM_DVE_SPECS[_VQARG_NAME] = spec
    _VQARG_OP = op
    return op


# ----------------------------------------------------------------------------
# device program

_PROG = None

import os as _os
ACT_COPY = _os.environ.get("VQ_ACT_COPY", "1") == "1"
N_WARMUP = int(_os.environ.get("VQ_WARMUP", "0"))


def build_program(vox_per_core=VOX_PER_CORE, chunk_v=CHUNK_V):
    """Build + compile the per-core SPMD Bass program. Returns (nc, meta)."""
    vqarg = _register_vqarg()
    dt = mybir.dt
    chunks = vox_per_core // chunk_v
    tiles_per_chunk = chunk_v // TILE_V
    n_tiles = vox_per_core // TILE_V

    nc = bacc.Bacc(
        "TRN2", target_bir_lowering=False, debug=False, num_devices=N_CORES
    )
    xh_d = nc.dram_tensor("xh", [C, vox_per_core], dt.bfloat16, kind="ExternalInput").ap()
    xl_d = nc.dram_tensor("xl", [C, vox_per_core], dt.bfloat16, kind="ExternalInput").ap()
    ph_d = nc.dram_tensor("pht", [C, K], dt.bfloat16, kind="ExternalInput").ap()
    pl_d = nc.dram_tensor("plt", [C, K], dt.bfloat16, kind="ExternalInput").ap()
    out_d = nc.dram_tensor("outA", [TILE_V, n_tiles], dt.float32, kind="ExternalOutput").ap()

    with tile.TileContext(nc) as tc, ExitStack() as ctx:
        cpool = ctx.enter_context(tc.tile_pool(name="const", bufs=1))
        xpool = ctx.enter_context(tc.tile_pool(name="x", bufs=3))
        ppool = ctx.enter_context(tc.tile_pool(name="psum", bufs=8, space="PSUM"))
        spool = ctx.enter_context(tc.tile_pool(name="scr", bufs=3))
        hpool = ctx.enter_context(tc.tile_pool(name="half", bufs=3))
        apool = ctx.enter_context(tc.tile_pool(name="acc", bufs=1))

        # tables go on the gpsimd DMA queue so they land in parallel with the
        # first x chunk on the sync queue (PE needs both before matmul 0)
        ph_sb = cpool.tile([C, K], dt.bfloat16)
        nc.gpsimd.dma_start(ph_sb[:], ph_d[:])
        pl_sb = cpool.tile([C, K], dt.bfloat16)
        nc.gpsimd.dma_start(pl_sb[:], pl_d[:])

        jsb = apool.tile([TILE_V, n_tiles], dt.float32)

        # PE warmup: dense matmuls on the (tiny, early-arriving) prototype
        # table while the first x chunk is still in flight. Releases the
        # HAM clock throttle (~3.4us of sustained PE activity -> 2.4 GHz)
        # before real work begins. Results are discarded.
        if N_WARMUP:
            # bridge PE activity from table-arrival to first-chunk-arrival so
            # the HAM clock gate releases sooner; sized to the DMA gap only.
            wps = ppool.tile([TILE_V, K], dt.float32, tag="ps")
            for _ in range(N_WARMUP):
                nc.tensor.matmul(wps[:], ph_sb[:, 0:TILE_V], ph_sb[:],
                                 start=True, stop=True)

        # ramp-in: small leading chunks so the first matmul starts sooner
        if chunks > 2:
            sizes = [256, 256, 512] + [chunk_v] * (chunks - 1)
        else:
            sizes = [chunk_v] * chunks
        assert sum(sizes) == vox_per_core
        base = 0
        tid = 0
        for cv in sizes:
            xh_sb = xpool.tile([C, cv], dt.bfloat16, tag="xh")
            nc.sync.dma_start(xh_sb[:], xh_d[:, base:base + cv])
            xl_sb = xpool.tile([C, cv], dt.bfloat16, tag="xl")
            nc.sync.dma_start(xl_sb[:], xl_d[:, base:base + cv])
            base += cv
            for t in range(cv // TILE_V):
                lhs_h = xh_sb[:, t * TILE_V:(t + 1) * TILE_V]
                lhs_l = xl_sb[:, t * TILE_V:(t + 1) * TILE_V]
                ps = ppool.tile([TILE_V, K], dt.float32)
                # lhs_h used twice consecutively (weight reuse), lhs_l last
                nc.tensor.matmul(ps[:], lhs_h, ph_sb[:], start=True, stop=False)
                nc.tensor.matmul(ps[:], lhs_h, pl_sb[:], start=False, stop=False)
                nc.tensor.matmul(ps[:], lhs_l, ph_sb[:], start=False, stop=True)
                scr = spool.tile([TILE_V, K // 2], dt.float32)
                if ACT_COPY:
                    # Scalar engine stages the second half into SBUF so the DVE
                    # op reads one PSUM stream + one SBUF stream.
                    half = hpool.tile([TILE_V, K // 2], dt.float32)
                    nc.scalar.copy(half[:], ps[:, K // 2:K])
                    in1 = half[:]
                else:
                    in1 = ps[:, K // 2:K]
                nc.vector._custom_dve(
                    vqarg,
                    out=scr[:],
                    in0=ps[:, 0:K // 2],
                    in1=in1,
                    s0=0.0,
                    s1=2.0,
                    imm2=1024.0,
                    accum_out=jsb[:, tid:tid + 1],
                )
                tid += 1
                if n_tiles > 32 and tid == n_tiles - 24:
                    # drain most results early (hidden under remaining tiles)
                    # so only a 12KB DMA sits after the last fold
                    nc.sync.dma_start(out_d[:, :tid], jsb[:, :tid])
        assert tid == n_tiles
        split = n_tiles - 24 if n_tiles > 32 else 0
        nc.sync.dma_start(out_d[:, split:], jsb[:, split:])

    nc.compile()
    return nc


def _get_program():
    global _PROG
    if _PROG is None:
        _PROG = build_program()
    return _PROG


# ----------------------------------------------------------------------------
# host-side prep + entry point

def _bf16_split(a):
    hi = a.astype(_BF16)
    lo = (a - hi.astype(np.float32)).astype(_BF16)
    return hi, lo


def _prep_prototypes(prototypes):
    pn = prototypes / np.maximum(
        np.linalg.norm(prototypes, axis=1, keepdims=True), 1e-12
    )
    pn = pn.astype(np.float32)
    q = np.arange(K // 2)
    perm = np.concatenate([511 - 2 * q, 510 - 2 * q])  # col layout for VQARG
    pc = pn[perm]
    ph, pl = _bf16_split(pc)
    pht = np.ascontiguousarray(ph.T)  # [96, 512] bf16
    plt = np.ascontiguousarray(pl.T)
    return pht, plt


def kernel(x, prototypes):
    x = np.asarray(x, np.float32)
    prototypes = np.asarray(prototypes, np.float32)

    # [2,96,48,48,48] -> [96, 221184] with global voxel = b*110592 + dhw
    xt = np.ascontiguousarray(
        x.reshape(B, C, D * H * W).transpose(1, 0, 2).reshape(C, N_VOX)
    )
    xh, xl = _bf16_split(xt)
    pht, plt = _prep_prototypes(prototypes)

    in_maps = []
    for c in range(N_CORES):
        sl = slice(c * VOX_PER_CORE, (c + 1) * VOX_PER_CORE)
        in_maps.append({
            "xh": np.ascontiguousarray(xh[:, sl]),
            "xl": np.ascontiguousarray(xl[:, sl]),
            "pht": pht,
            "plt": plt,
        })

    nc = _get_program()
    res = None
    last_err = None
    for attempt in range(3):
        try:
            res = run_bass_kernel_spmd(nc, in_maps, list(range(N_CORES)))
            break
        except Exception as e:  # transient axon/NRT hiccups self-recover
            last_err = e
            import time as _time
            _time.sleep(20 * (attempt + 1))
    if res is None:
        raise last_err

    outs = []
    for c in range(N_CORES):
        A = np.asarray(res.results[c]["outA"], np.float32)  # [128, 216]
        kidx = -(A + np.float32(511.0))                     # exact small ints
        outs.append(kidx.T.reshape(-1))                     # voxel = t*128 + p
    full = np.concatenate(outs)
    return full.reshape(B, D, H, W).astype(np.int32)

